# revision 15
# baseline (speedup 1.0000x reference)
"""MoE dispatch/combine kernel for Trainium2 (8 NeuronCores, token-parallel).

Computes, for hidden_states [B=4, S=4096, H=2048], router_weight [E=64, H],
router_bias [E], expert_bias [E, H], TOP_K=8:

    logits = x @ rw.T + rb ; scores = softmax(logits) ; top8
    out = x * (sum top8 scores) + (top8-masked scores) @ expert_bias

Fully *transposed* dataflow (per core: 2048 tokens, no collectives):
  - Host supplies x already transposed + fp16: xt[p, g, c, i] = x[512g+i, 128c+p]
    (g: 4 groups of 512 tokens, c: 16 h-chunks). One 8.4MB input copy; 8.4MB
    fp16 transposed output; host undoes the transpose. Total HBM traffic per
    core ~17MB vs ~26MB for the untransposed design -- and the PE never
    transposes x (the old kernel spent ~40% of PE time on x transposes).
  - Router: logitsT[64, t] accumulates rw_chunk.T @ xt_chunk (rw stationary).
  - Small PE transposes move logits to [t, e] for the DVE Max8 top-8 path;
    masked normalized scores c[t, e] (fp16) transpose back to cT[e, t].
  - a[t] = sum_e c[t, e] materializes *broadcast across partitions* with one
    ones[64,128].T @ cT matmul -- no per-token reduction op needed.
  - Combine: outT[h-chunk, t] = eb_chunk.T @ cT (eb is a natural lhsT in
    [E, H] layout -- no transpose), drained by DVE as psum + a*xt in one
    tensor_tensor over a precomputed axt = xt * a_bcast tile.

fp16 x/router: logits have std ~45 so softmax is extremely peaked; fp16
logit noise ~0.03 only reorders near-tied experts (harmless). fp16 x on the
dominant x*a term gives ~3e-4 relative error (gate is 2e-2).
"""
import os
import sys

for _p in ("/opt/trn_rl_repo", "/opt/pypackages"):
    if _p not in sys.path:
        sys.path.append(_p)

os.environ.setdefault("BASS_NEVER_TRACE", "1")

import numpy as np
from contextlib import ExitStack

import concourse.bass as bass
import concourse.tile as tile
from concourse import bacc, mybir
from concourse.bass_utils import run_bass_kernel_spmd

F32 = mybir.dt.float32
F16 = mybir.dt.float16
AF = mybir.ActivationFunctionType
AL = mybir.AluOpType

B, S, H, E, TOPK = 4, 4096, 2048, 64, 8
T = B * S
N_CORES = 8
T_PC = T // N_CORES            # 2048 tokens per core
NG = 4                         # token groups per core
GT = T_PC // NG                # 512 tokens per group
NTIL = GT // 128               # 4 token tiles per group
HCH = H // 128                 # 16 h-chunks
CPB = 2                        # h-chunks per output psum tile (2 banks)


def _build():
    nc = bacc.Bacc("TRN2", target_bir_lowering=False, debug=False,
                   num_devices=N_CORES)

    # xt[p, g, c, i] = x[t=512g+i, h=128c+p], fp16, flat [128, NG*HCH*GT]
    xt_d = nc.dram_tensor("xt", [128, NG * HCH * GT], F16,
                          kind="ExternalInput").ap()
    # rwt[p, c*E+e] = rw[e, 128c+p]
    rwt_d = nc.dram_tensor("rwt", [128, HCH * E], F16, kind="ExternalInput").ap()
    eb_d = nc.dram_tensor("eb", [E, H], F16, kind="ExternalInput").ap()
    rb_d = nc.dram_tensor("rb", [E, 1], F32, kind="ExternalInput").ap()
    idf_d = nc.dram_tensor("idf", [128, 128], F32, kind="ExternalInput").ap()
    idh_d = nc.dram_tensor("idh", [128, 128], F16, kind="ExternalInput").ap()
    ones_d = nc.dram_tensor("ones", [E, 128], F16, kind="ExternalInput").ap()
    # out[p, g, c, i] = out[t=512g+i, h=128c+p], fp16
    out_d = nc.dram_tensor("out", [128, NG * HCH * GT], F16,
                           kind="ExternalOutput").ap()

    with tile.TileContext(nc) as tc:
        with ExitStack() as ctx:
            consts = ctx.enter_context(tc.tile_pool(name="consts", bufs=1))
            lgsp = ctx.enter_context(tc.tile_pool(name="lgsp", bufs=2))
            wsb = ctx.enter_context(tc.tile_pool(name="wsb", bufs=2))
            stp = ctx.enter_context(tc.tile_pool(name="stp", bufs=3))
            ctp = ctx.enter_context(tc.tile_pool(name="ctp", bufs=2))
            abp = ctx.enter_context(tc.tile_pool(name="abp", bufs=2))
            axp = ctx.enter_context(tc.tile_pool(name="axp", bufs=2))
            osb = ctx.enter_context(tc.tile_pool(name="osb", bufs=8))

            lg_ps = ctx.enter_context(
                tc.tile_pool(name="lg_ps", bufs=1, space="PSUM"))
            w_ps = ctx.enter_context(
                tc.tile_pool(name="w_ps", bufs=1, space="PSUM"))
            ct_ps = w_ps
            out_ps = ctx.enter_context(
                tc.tile_pool(name="out_ps", bufs=2, space="PSUM"))

            # ---- constants + x prefetch, ordered for fast router start:
            # rwt first, then group-0 x in quarter slabs, then the remaining
            # consts, then groups 1-3 in half slabs ----
            rwt = consts.tile([128, HCH, E], F16)
            nc.scalar.dma_start(rwt[:].rearrange("p c e -> p (c e)"), rwt_d)
            xt = consts.tile([128, NG, HCH, GT], F16)

            def xt_load(g, c0, nch, eng=None):
                lo = (g * HCH + c0) * GT
                (eng or nc.sync).dma_start(
                    xt[:, g, c0:c0 + nch, :].rearrange("p c i -> p (c i)"),
                    xt_d[:, lo:lo + nch * GT])

            for c0 in range(0, HCH, 4):
                xt_load(0, c0, 4, eng=nc.scalar if (c0 // 4) % 2 == 0 else None)
            idh = consts.tile([128, 128], F16)
            nc.sync.dma_start(idh[:], idh_d)
            idf = consts.tile([128, 128], F32)
            nc.sync.dma_start(idf[:], idf_d)
            rb = consts.tile([E, 1], F32)
            nc.sync.dma_start(rb[:], rb_d)
            ones = consts.tile([E, 128], F16)
            nc.sync.dma_start(ones[:], ones_d)
            eb = consts.tile([E, H], F16)
            nc.sync.dma_start(eb[:], eb_d)
            for g in range(1, NG):
                xt_load(g, 0, HCH)

            # Software pipeline: emit group g-1's combine between group g's
            # router and softmax.  The PE instruction queue is FIFO, so this
            # ordering keeps the PE streaming real (HAM-warming) matmuls while
            # DVE/ACT run group g's softmax -- without it the PE idles >3.4us
            # every group, the HAM clock gate re-throttles to 1.2 GHz, and
            # every matmul runs at half speed.
            ot_state = {}

            def emit_combine_pair(g, cT, axt, c0):
                # two pairs share one ot tile so out-DMAs are 512KB not 256KB
                half = (c0 // CPB) % 2
                if half == 0:
                    ot_state['t'] = osb.tile([128, 2 * CPB, GT], F16, tag="ot", name=f"ot_{g}_{c0}")
                ot = ot_state['t']
                ops_ = out_ps.tile([128, CPB, GT], F32, tag="ops")
                for k in range(CPB):
                    c = c0 + k
                    nc.tensor.matmul(ops_[:, k, :],
                                     eb[:, 128 * c:128 * (c + 1)], cT[:],
                                     start=True, stop=True)
                comb = osb.tile([128, CPB, GT], F16, tag="comb")
                nc.scalar.copy(comb[:], ops_[:])
                nc.vector.tensor_tensor(ot[:, half * CPB:(half + 1) * CPB, :],
                                        comb[:],
                                        axt[:, c0:c0 + CPB, :], op=AL.add)
                if half == 1:
                    lo = (g * HCH + c0 - CPB) * GT
                    nc.sync.dma_start(
                        out_d[:, lo:lo + 2 * CPB * GT],
                        ot[:].rearrange("p k i -> p (k i)"))

            def emit_router(g):
                # logitsT [E, GT] accumulation + bias drain
                lg = lg_ps.tile([E, GT], F32, tag="lg")
                for c in range(HCH):
                    nc.tensor.matmul(lg[:], rwt[:, c, :], xt[:, g, c, :],
                                     start=(c == 0), stop=(c == HCH - 1))
                lgs = lgsp.tile([E, GT], F32, tag="lgs")
                nc.scalar.activation(lgs[:], lg[:], AF.Identity,
                                     bias=rb[:], scale=1.0)
                return lgs

            # prologue: group 0's router
            lgs_cur = emit_router(0)

            prev = None
            for g in range(NG):
                # ---- logits to [token, expert] (4 tiles of 128 tokens) ----
                wps = w_ps.tile([128, NTIL, E], F32, tag="wps")
                for i in range(NTIL):
                    nc.tensor.matmul(
                        wps[:, i, :], lgs_cur[:, 128 * i:128 * (i + 1)],
                        idf[0:E, 0:E], is_transpose=True,
                        start=True, stop=True)
                w = wsb.tile([128, NTIL, E], F32, tag="w")
                nc.scalar.copy(w[:], wps[:])

                # ---- softmax tiles + next group's router interleaved with the
                # previous group's combine c-pairs: keeps the PE streaming
                # (HAM warm) while not queueing all 8 ACT psum-drains ahead of
                # the exps ----
                ctps = ct_ps.tile([E, NTIL, 128], F16, tag="ctps")

                def softmax_tile(i):
                    top8 = stp.tile([128, TOPK], F32, tag=f"top8_{i}")
                    nc.vector.max(top8[:], w[:, i, :])
                    negm = stp.tile([128, 1], F32, tag=f"negm_{i}")
                    nc.vector.tensor_scalar(negm[:], top8[:, 0:1], -1.0, None,
                                            AL.mult)
                    y = stp.tile([128, E], F32, tag=f"y_{i}")
                    z = stp.tile([128, 1], F32, tag=f"z_{i}")
                    nc.scalar.activation(y[:], w[:, i, :], AF.Exp,
                                         bias=negm[:], scale=1.0,
                                         accum_out=z[:])
                    iz = stp.tile([128, 1], F32, tag=f"iz_{i}")
                    nc.vector.reciprocal(iz[:], z[:])
                    g01 = stp.tile([128, E], F32, tag=f"g01_{i}")
                    nc.vector.tensor_scalar(g01[:], w[:, i, :],
                                            top8[:, TOPK - 1:TOPK],
                                            iz[:], AL.is_ge, AL.mult)
                    cmask = stp.tile([128, E], F16, tag=f"c_{i}")
                    nc.vector.tensor_tensor(cmask[:], y[:], g01[:], op=AL.mult)
                    nc.tensor.matmul(ctps[:, i, :], cmask[:], idh[:],
                                     is_transpose=True, start=True, stop=True)

                if prev is not None:
                    pg, pcT, paxt = prev
                    for j in range(HCH // CPB):
                        emit_combine_pair(pg, pcT, paxt, j * CPB)
                        if j < NTIL:
                            softmax_tile(j)
                else:
                    for i in range(NTIL):
                        softmax_tile(i)
                # next group's router rides the same PE stream (its lg bank is
                # free once lgs_cur drained; its xt slab is prefetched)
                if g + 1 < NG:
                    lgs_cur = emit_router(g + 1)
                cT = ctp.tile([E, NTIL * 128], F16, tag="cT")
                nc.scalar.copy(cT[:], ctps[:].rearrange("e n p -> e (n p)"))

                # ---- a[t] broadcast across partitions: ones.T @ cT ----
                abps = w_ps.tile([128, GT], F32, tag="abps")
                nc.tensor.matmul(abps[:], ones[:], cT[:], start=True, stop=True)
                ab = abp.tile([128, GT], F16, tag="ab")
                nc.scalar.copy(ab[:], abps[:])

                # ---- axt = xt * a (DVE 2x fp16; gpsimd is slower and its
                # SBUF traffic stalls concurrent DVE ops) ----
                axt = axp.tile([128, HCH, GT], F16, tag="axt")
                ab_bc4 = ab[:].unsqueeze(1).broadcast_to((128, 4, GT))
                for c0 in range(0, HCH, 4):
                    nc.vector.tensor_tensor(axt[:, c0:c0 + 4, :],
                                            xt[:, g, c0:c0 + 4, :],
                                            ab_bc4, op=AL.mult)

                prev = (g, cT, axt)

            pg, pcT, paxt = prev
            for j in range(HCH // CPB):
                emit_combine_pair(pg, pcT, paxt, j * CPB)

    nc.compile()
    return nc


_NC_CACHE = None


def _get_nc():
    global _NC_CACHE
    if _NC_CACHE is None:
        _NC_CACHE = _build()
    return _NC_CACHE


def _prep_inputs(hidden_states, router_weight, router_bias, expert_bias):
    flat = np.ascontiguousarray(hidden_states.reshape(T, H), dtype=np.float32)
    rwt = np.ascontiguousarray(
        router_weight.T.reshape(HCH, 128, E).transpose(1, 0, 2).reshape(128, HCH * E)
    ).astype(np.float16)
    rb = np.ascontiguousarray(router_bias.reshape(E, 1)).astype(np.float32)
    eb = np.ascontiguousarray(expert_bias).astype(np.float16)
    eye = np.eye(128, dtype=np.float32)
    eye_h = eye.astype(np.float16)
    ones = np.ones((E, 128), dtype=np.float16)
    in_maps = []
    for cc in range(N_CORES):
        xc = flat[cc * T_PC:(cc + 1) * T_PC]              # [2048t, 2048h]
        xcT = np.ascontiguousarray(xc.T).astype(np.float16)   # [2048h, 2048t]
        # [h, t] -> [p, g, c, i]: h = 128c + p, t = 512g + i
        xt = np.ascontiguousarray(
            xcT.reshape(HCH, 128, NG, GT).transpose(1, 2, 0, 3)
        ).reshape(128, NG * HCH * GT)
        in_maps.append({
            "xt": xt,
            "rwt": rwt,
            "eb": eb,
            "rb": rb,
            "idf": eye,
            "idh": eye_h,
            "ones": ones,
        })
    return in_maps


def kernel(hidden_states, router_weight, router_bias, expert_bias):
    hidden_states = np.asarray(hidden_states, dtype=np.float32)
    router_weight = np.asarray(router_weight, dtype=np.float32)
    router_bias = np.asarray(router_bias, dtype=np.float32)
    expert_bias = np.asarray(expert_bias, dtype=np.float32)
    assert hidden_states.shape == (B, S, H)

    nc = _get_nc()
    in_maps = _prep_inputs(hidden_states, router_weight, router_bias, expert_bias)
    res = run_bass_kernel_spmd(nc, in_maps, list(range(N_CORES)))
    out = np.empty((T, H), dtype=np.float32)
    for cc in range(N_CORES):
        arr = np.asarray(res.results[cc]["out"]).reshape(128, NG, HCH, GT)
        # [p, g, c, i] -> [t, h]
        out[cc * T_PC:(cc + 1) * T_PC] = (
            arr.transpose(1, 3, 2, 0).reshape(T_PC, H).astype(np.float32))
    return out.reshape(B, S, H)


if __name__ == "__main__":
    rng = np.random.default_rng(0)
    hs = rng.standard_normal((B, S, H), dtype=np.float32)
    rw = rng.standard_normal((E, H), dtype=np.float32)
    rbv = np.zeros((E,), dtype=np.float32)
    ebv = (rng.standard_normal((E, H), dtype=np.float32) * 0.1).astype(np.float32)
    o = kernel(hidden_states=hs, router_weight=rw, router_bias=rbv, expert_bias=ebv)
    print("kernel out", o.shape, o.dtype, float(np.abs(o).mean()))


# revision 16
# speedup vs baseline: 1.0209x; 1.0209x over previous
"""MoE dispatch/combine kernel for Trainium2 (8 NeuronCores, token-parallel).

Computes, for hidden_states [B=4, S=4096, H=2048], router_weight [E=64, H],
router_bias [E], expert_bias [E, H], TOP_K=8:

    logits = x @ rw.T + rb ; scores = softmax(logits) ; top8
    out = x * (sum top8 scores) + (top8-masked scores) @ expert_bias

Fully *transposed* dataflow (per core: 2048 tokens, no collectives):
  - Host supplies x already transposed + fp16: xt[p, g, c, i] = x[512g+i, 128c+p]
    (g: 4 groups of 512 tokens, c: 16 h-chunks). One 8.4MB input copy; 8.4MB
    fp16 transposed output; host undoes the transpose. Total HBM traffic per
    core ~17MB vs ~26MB for the untransposed design -- and the PE never
    transposes x (the old kernel spent ~40% of PE time on x transposes).
  - Router: logitsT[64, t] accumulates rw_chunk.T @ xt_chunk (rw stationary).
  - Small PE transposes move logits to [t, e] for the DVE Max8 top-8 path;
    masked normalized scores c[t, e] (fp16) transpose back to cT[e, t].
  - a[t] = sum_e c[t, e] materializes *broadcast across partitions* with one
    ones[64,128].T @ cT matmul -- no per-token reduction op needed.
  - Combine: outT[h-chunk, t] = eb_chunk.T @ cT (eb is a natural lhsT in
    [E, H] layout -- no transpose), drained by DVE as psum + a*xt in one
    tensor_tensor over a precomputed axt = xt * a_bcast tile.

fp16 x/router: logits have std ~45 so softmax is extremely peaked; fp16
logit noise ~0.03 only reorders near-tied experts (harmless). fp16 x on the
dominant x*a term gives ~3e-4 relative error (gate is 2e-2).
"""
import os
import sys

for _p in ("/opt/trn_rl_repo", "/opt/pypackages"):
    if _p not in sys.path:
        sys.path.append(_p)

os.environ.setdefault("BASS_NEVER_TRACE", "1")

import numpy as np
from contextlib import ExitStack

import concourse.bass as bass
import concourse.tile as tile
from concourse import bacc, mybir
from concourse.bass_utils import run_bass_kernel_spmd

F32 = mybir.dt.float32
F16 = mybir.dt.float16
AF = mybir.ActivationFunctionType
AL = mybir.AluOpType

B, S, H, E, TOPK = 4, 4096, 2048, 64, 8
T = B * S
N_CORES = 8
T_PC = T // N_CORES            # 2048 tokens per core
NG = 4                         # token groups per core
GT = T_PC // NG                # 512 tokens per group
NTIL = GT // 128               # 4 token tiles per group
HCH = H // 128                 # 16 h-chunks
CPB = 2                        # h-chunks per output psum tile (2 banks)


def _build():
    nc = bacc.Bacc("TRN2", target_bir_lowering=False, debug=False,
                   num_devices=N_CORES)

    # xt[p, g, c, i] = x[t=512g+i, h=128c+p], fp16, flat [128, NG*HCH*GT]
    xt_d = nc.dram_tensor("xt", [128, NG * HCH * GT], F16,
                          kind="ExternalInput").ap()
    # rwt[p, c*E+e] = rw[e, 128c+p]
    rwt_d = nc.dram_tensor("rwt", [128, HCH * E], F16, kind="ExternalInput").ap()
    eb_d = nc.dram_tensor("eb", [E, H], F16, kind="ExternalInput").ap()
    rb_d = nc.dram_tensor("rb", [E, 1], F32, kind="ExternalInput").ap()
    idf_d = nc.dram_tensor("idf", [128, 128], F32, kind="ExternalInput").ap()
    idh_d = nc.dram_tensor("idh", [128, 128], F16, kind="ExternalInput").ap()
    ones_d = nc.dram_tensor("ones", [E, 128], F16, kind="ExternalInput").ap()
    # out[p, g, c, i] = out[t=512g+i, h=128c+p], fp16
    out_d = nc.dram_tensor("out", [128, NG * HCH * GT], F16,
                           kind="ExternalOutput").ap()

    with tile.TileContext(nc) as tc:
        with ExitStack() as ctx:
            consts = ctx.enter_context(tc.tile_pool(name="consts", bufs=1))
            lgsp = ctx.enter_context(tc.tile_pool(name="lgsp", bufs=2))
            wsb = ctx.enter_context(tc.tile_pool(name="wsb", bufs=2))
            stp = ctx.enter_context(tc.tile_pool(name="stp", bufs=3))
            ctp = ctx.enter_context(tc.tile_pool(name="ctp", bufs=2))
            abp = ctx.enter_context(tc.tile_pool(name="abp", bufs=2))
            axp = ctx.enter_context(tc.tile_pool(name="axp", bufs=2))
            osb = ctx.enter_context(tc.tile_pool(name="osb", bufs=8))

            lg_ps = ctx.enter_context(
                tc.tile_pool(name="lg_ps", bufs=1, space="PSUM"))
            w_ps = ctx.enter_context(
                tc.tile_pool(name="w_ps", bufs=1, space="PSUM"))
            ct_ps = w_ps
            out_ps = ctx.enter_context(
                tc.tile_pool(name="out_ps", bufs=2, space="PSUM"))

            # ---- constants + x prefetch, ordered for fast router start:
            # rwt first, then group-0 x in quarter slabs, then the remaining
            # consts, then groups 1-3 in half slabs ----
            rwt = consts.tile([128, HCH, E], F16)
            nc.scalar.dma_start(rwt[:].rearrange("p c e -> p (c e)"), rwt_d)
            xt = consts.tile([128, NG, HCH, GT], F16)

            def xt_load(g, c0, nch, eng=None):
                lo = (g * HCH + c0) * GT
                (eng or nc.sync).dma_start(
                    xt[:, g, c0:c0 + nch, :].rearrange("p c i -> p (c i)"),
                    xt_d[:, lo:lo + nch * GT])

            for c0 in range(0, HCH, 4):
                xt_load(0, c0, 4, eng=nc.scalar if (c0 // 4) % 2 == 0 else None)
            idh = consts.tile([128, 128], F16)
            nc.sync.dma_start(idh[:], idh_d)
            idf = consts.tile([128, 128], F32)
            nc.sync.dma_start(idf[:], idf_d)
            rb = consts.tile([E, 1], F32)
            nc.sync.dma_start(rb[:], rb_d)
            ones = consts.tile([E, 128], F16)
            nc.sync.dma_start(ones[:], ones_d)
            eb = consts.tile([E, H], F16)
            nc.sync.dma_start(eb[:], eb_d)
            for g in range(1, NG):
                xt_load(g, 0, HCH)

            # Software pipeline: emit group g-1's combine between group g's
            # router and softmax.  The PE instruction queue is FIFO, so this
            # ordering keeps the PE streaming real (HAM-warming) matmuls while
            # DVE/ACT run group g's softmax -- without it the PE idles >3.4us
            # every group, the HAM clock gate re-throttles to 1.2 GHz, and
            # every matmul runs at half speed.
            def emit_combine_pair(g, cT, axt, c0):
                ops_ = out_ps.tile([128, CPB, GT], F32, tag="ops")
                for k in range(CPB):
                    c = c0 + k
                    nc.tensor.matmul(ops_[:, k, :],
                                     eb[:, 128 * c:128 * (c + 1)], cT[:],
                                     start=True, stop=True)
                comb = osb.tile([128, CPB, GT], F16, tag="comb")
                nc.scalar.copy(comb[:], ops_[:])
                ot = osb.tile([128, CPB, GT], F16, tag="ot")
                nc.vector.tensor_tensor(ot[:], comb[:],
                                        axt[:, c0:c0 + CPB, :], op=AL.add)
                nc.sync.dma_start(
                    out_d[:, (g * HCH + c0) * GT:(g * HCH + c0 + CPB) * GT],
                    ot[:].rearrange("p k i -> p (k i)"))

            def emit_router(g):
                # logitsT [E, GT] accumulation + bias drain
                lg = lg_ps.tile([E, GT], F32, tag="lg")
                for c in range(HCH):
                    nc.tensor.matmul(lg[:], rwt[:, c, :], xt[:, g, c, :],
                                     start=(c == 0), stop=(c == HCH - 1))
                lgs = lgsp.tile([E, GT], F32, tag="lgs")
                nc.scalar.activation(lgs[:], lg[:], AF.Identity,
                                     bias=rb[:], scale=1.0)
                return lgs

            # prologue: group 0's router
            lgs_cur = emit_router(0)

            prev = None
            for g in range(NG):
                # ---- logits to [token, expert] (4 tiles of 128 tokens) ----
                wps = w_ps.tile([128, NTIL, E], F32, tag="wps")
                for i in range(NTIL):
                    nc.tensor.matmul(
                        wps[:, i, :], lgs_cur[:, 128 * i:128 * (i + 1)],
                        idf[0:E, 0:E], is_transpose=True,
                        start=True, stop=True)
                w = wsb.tile([128, NTIL, E], F32, tag="w")
                nc.scalar.copy(w[:], wps[:])

                # ---- softmax tiles + next group's router interleaved with the
                # previous group's combine c-pairs: keeps the PE streaming
                # (HAM warm) while not queueing all 8 ACT psum-drains ahead of
                # the exps ----
                ctps = ct_ps.tile([E, NTIL, 128], F16, tag="ctps")

                def softmax_tile(i):
                    top8 = stp.tile([128, TOPK], F32, tag=f"top8_{i}")
                    nc.vector.max(top8[:], w[:, i, :])
                    negm = stp.tile([128, 1], F32, tag=f"negm_{i}")
                    nc.vector.tensor_scalar(negm[:], top8[:, 0:1], -1.0, None,
                                            AL.mult)
                    y = stp.tile([128, E], F32, tag=f"y_{i}")
                    z = stp.tile([128, 1], F32, tag=f"z_{i}")
                    nc.scalar.activation(y[:], w[:, i, :], AF.Exp,
                                         bias=negm[:], scale=1.0,
                                         accum_out=z[:])
                    iz = stp.tile([128, 1], F32, tag=f"iz_{i}")
                    nc.vector.reciprocal(iz[:], z[:])
                    g01 = stp.tile([128, E], F32, tag=f"g01_{i}")
                    nc.vector.tensor_scalar(g01[:], w[:, i, :],
                                            top8[:, TOPK - 1:TOPK],
                                            iz[:], AL.is_ge, AL.mult)
                    cmask = stp.tile([128, E], F16, tag=f"c_{i}")
                    nc.vector.tensor_tensor(cmask[:], y[:], g01[:], op=AL.mult)
                    nc.tensor.matmul(ctps[:, i, :], cmask[:], idh[:],
                                     is_transpose=True, start=True, stop=True)

                if prev is not None:
                    pg, pcT, paxt = prev
                    for j in range(HCH // CPB):
                        emit_combine_pair(pg, pcT, paxt, j * CPB)
                        if j < NTIL:
                            softmax_tile(j)
                else:
                    for i in range(NTIL):
                        softmax_tile(i)
                # next group's router rides the same PE stream (its lg bank is
                # free once lgs_cur drained; its xt slab is prefetched)
                if g + 1 < NG:
                    lgs_cur = emit_router(g + 1)
                cT = ctp.tile([E, NTIL * 128], F16, tag="cT")
                nc.scalar.copy(cT[:], ctps[:].rearrange("e n p -> e (n p)"))

                # ---- a[t] broadcast across partitions: ones.T @ cT ----
                abps = w_ps.tile([128, GT], F32, tag="abps")
                nc.tensor.matmul(abps[:], ones[:], cT[:], start=True, stop=True)
                ab = abp.tile([128, GT], F16, tag="ab")
                nc.scalar.copy(ab[:], abps[:])

                # ---- axt = xt * a (DVE 2x fp16; gpsimd is slower and its
                # SBUF traffic stalls concurrent DVE ops) ----
                axt = axp.tile([128, HCH, GT], F16, tag="axt")
                ab_bc4 = ab[:].unsqueeze(1).broadcast_to((128, 4, GT))
                for c0 in range(0, HCH, 4):
                    nc.vector.tensor_tensor(axt[:, c0:c0 + 4, :],
                                            xt[:, g, c0:c0 + 4, :],
                                            ab_bc4, op=AL.mult)

                prev = (g, cT, axt)

            pg, pcT, paxt = prev
            for j in range(HCH // CPB):
                emit_combine_pair(pg, pcT, paxt, j * CPB)

    nc.compile()
    return nc


_NC_CACHE = None


def _get_nc():
    global _NC_CACHE
    if _NC_CACHE is None:
        _NC_CACHE = _build()
    return _NC_CACHE


def _prep_inputs(hidden_states, router_weight, router_bias, expert_bias):
    flat = np.ascontiguousarray(hidden_states.reshape(T, H), dtype=np.float32)
    rwt = np.ascontiguousarray(
        router_weight.T.reshape(HCH, 128, E).transpose(1, 0, 2).reshape(128, HCH * E)
    ).astype(np.float16)
    rb = np.ascontiguousarray(router_bias.reshape(E, 1)).astype(np.float32)
    eb = np.ascontiguousarray(expert_bias).astype(np.float16)
    eye = np.eye(128, dtype=np.float32)
    eye_h = eye.astype(np.float16)
    ones = np.ones((E, 128), dtype=np.float16)
    in_maps = []
    for cc in range(N_CORES):
        xc = flat[cc * T_PC:(cc + 1) * T_PC]              # [2048t, 2048h]
        xcT = np.ascontiguousarray(xc.T).astype(np.float16)   # [2048h, 2048t]
        # [h, t] -> [p, g, c, i]: h = 128c + p, t = 512g + i
        xt = np.ascontiguousarray(
            xcT.reshape(HCH, 128, NG, GT).transpose(1, 2, 0, 3)
        ).reshape(128, NG * HCH * GT)
        in_maps.append({
            "xt": xt,
            "rwt": rwt,
            "eb": eb,
            "rb": rb,
            "idf": eye,
            "idh": eye_h,
            "ones": ones,
        })
    return in_maps


def kernel(hidden_states, router_weight, router_bias, expert_bias):
    hidden_states = np.asarray(hidden_states, dtype=np.float32)
    router_weight = np.asarray(router_weight, dtype=np.float32)
    router_bias = np.asarray(router_bias, dtype=np.float32)
    expert_bias = np.asarray(expert_bias, dtype=np.float32)
    assert hidden_states.shape == (B, S, H)

    nc = _get_nc()
    in_maps = _prep_inputs(hidden_states, router_weight, router_bias, expert_bias)
    res = run_bass_kernel_spmd(nc, in_maps, list(range(N_CORES)))
    out = np.empty((T, H), dtype=np.float32)
    for cc in range(N_CORES):
        arr = np.asarray(res.results[cc]["out"]).reshape(128, NG, HCH, GT)
        # [p, g, c, i] -> [t, h]
        out[cc * T_PC:(cc + 1) * T_PC] = (
            arr.transpose(1, 3, 2, 0).reshape(T_PC, H).astype(np.float32))
    return out.reshape(B, S, H)


if __name__ == "__main__":
    rng = np.random.default_rng(0)
    hs = rng.standard_normal((B, S, H), dtype=np.float32)
    rw = rng.standard_normal((E, H), dtype=np.float32)
    rbv = np.zeros((E,), dtype=np.float32)
    ebv = (rng.standard_normal((E, H), dtype=np.float32) * 0.1).astype(np.float32)
    o = kernel(hidden_states=hs, router_weight=rw, router_bias=rbv, expert_bias=ebv)
    print("kernel out", o.shape, o.dtype, float(np.abs(o).mean()))
